# revision 13
# baseline (speedup 1.0000x reference)
"""CVKAN layer Trainium2 kernel (v4: single-act-table Gaussian cascade).

Math (per reference):
    u = (x + 1)/h,  h = 2/(NB-1) = 2/7
    basis_j(x) = exp(-(u - j)^2)                  j = 0..7, per part (re|im)
    out[b, o]  = sum_{p=(part,i), j} basis_j(x_p[b]) * w[p, j, o] + bias[o]

Single-seed cascade, all activations from ONE table (exp_and_others:
Square, Exp, Copy) so there are zero activation-table reloads:
    sq    = Square(x/h + 1/h)          = u^2            (fp32)
    b'_0  = Exp(-sq + C)               = e^{C - u^2}    (bf16, C=30 boost)
    r     = Exp(2x/h + 2/h)            = e^{2u}         (bf16)
    b'_j  = b'_{j-1} * r                                (bf16 multiplies)
    basis_j = b'_j * e^{-j^2 - C}      (constant folded into the weights)

Device strategy (8 cores, pure batch data-parallel, no collectives):
  - Host concatenates re|im into x2 [B_CORE, 128] bf16; XBAR dma_start_transpose
    loads T [128, chunk] directly (no PE transposes).
  - ScalarE: 3 passes per chunk (sq, seed, r) + the PSUM->SBUF eviction copy.
  - 7 cascade multiplies per chunk, column-striped: DVE takes 13/16 of the
    columns (bf16 2x_1p mode ~0.52 ns/col), Pool/GpSimd the rest.
  - PE: j-major accumulating matmuls with the basis block [128,128] as the
    STATIONARY operand and W_j [128, 32] moving: 32 output rows per matmul,
    and after the last cascade link only the j=7 matmuls remain (short tail).
    PSUM collects out[b, o] directly - no output transpose.
  - Evictions are software-pipelined one chunk behind the activations so the
    ScalarE queue never stalls on matmul completion.
"""

import sys

import numpy as np

if "/opt/trn_rl_repo" not in sys.path:
    sys.path.append("/opt/trn_rl_repo")

B = 65536
IN = 64
OUT = 16
NB = 8
N_CORES = 8
B_CORE = B // N_CORES  # 8192
H = 2.0 / (NB - 1)
CBOOST = 30.0

# Graduated chunk sizes (batch rows per pipeline stage). nblk = cn/128 must
# stay <= 16 so each chunk's PSUM tile [128, nblk*32] f32 fits one 2KB bank.
# Small first chunk shortens the pipeline fill (T-DMA -> sq -> seed -> links);
# smaller last chunk shortens the cascade->evict->store tail.
CHUNKS = [512, 2048, 2048, 2048, 1536]
assert sum(CHUNKS) == B_CORE
# Fraction of link columns the DVE takes (rest on Pool), in 128-col blocks.
DVE_FRAC = 13.0 / 16.0

_CACHE = {}


def _build_module():
    import concourse.mybir as mybir
    import concourse.tile as tile
    from concourse import bacc

    f32 = mybir.dt.float32
    bf16 = mybir.dt.bfloat16
    nc = bacc.Bacc("TRN2", target_bir_lowering=False, debug=False,
                   num_devices=N_CORES)

    x2 = nc.dram_tensor("x2", [B_CORE, 128], bf16, kind="ExternalInput")
    w = nc.dram_tensor("w", [128, NB * 2 * OUT], bf16, kind="ExternalInput")
    y = nc.dram_tensor("y", [B_CORE, 2 * OUT], f32, kind="ExternalOutput")

    Square = mybir.ActivationFunctionType.Square
    Exp = mybir.ActivationFunctionType.Exp
    Copy = mybir.ActivationFunctionType.Copy
    Mult = mybir.AluOpType.mult

    with tile.TileContext(nc) as tc:
        with (
            tc.tile_pool(name="consts", bufs=1) as consts,
            tc.tile_pool(name="tin", bufs=2) as tpool,
            tc.tile_pool(name="sq", bufs=2) as spool,
            tc.tile_pool(name="basis", bufs=2) as bpool,
            tc.tile_pool(name="opsum", bufs=1, space="PSUM") as opsum,
            tc.tile_pool(name="osb", bufs=2) as opool,
        ):
            # Per-partition bias columns (floats need pre-registered consts).
            gbias = consts.tile([128, 4], f32)
            nc.vector.memset(gbias[:, 0:1], 1.0 / H)
            nc.vector.memset(gbias[:, 1:2], CBOOST)
            nc.vector.memset(gbias[:, 2:3], 2.0 / H)
            nc.vector.memset(gbias[:, 3:4], 0.0)
            # Dummy activation: forces the one-and-only act-table load to run
            # at t~0, overlapped with the first input DMA, instead of sitting
            # on the critical path of the first chunk's activations.
            warm = consts.tile([128, 1], bf16)
            nc.scalar.activation(warm[:], gbias[:, 3:4], Exp,
                                 bias=gbias[:, 3:4])
            w_sb = consts.tile([128, NB * 2 * OUT], bf16)

            pending = []  # (psum, nblk, base) awaiting evict + store

            def flush_one():
                ps, nblk, base0 = pending.pop(0)
                out_sb = opool.tile([128, nblk * 2 * OUT], f32, tag="out_sb")
                nc.scalar.activation(out_sb[:], ps[:], Copy)
                nc.sync.dma_start(
                    out=y.ap()[base0:base0 + nblk * 128, :]
                        .rearrange("(g p) o -> p g o", p=128),
                    in_=out_sb[:].rearrange("p (g o) -> p g o", g=nblk),
                )

            base = 0
            for g, cn in enumerate(CHUNKS):
                nblk = cn // 128
                T = tpool.tile([128, cn], bf16, tag="T")
                nc.sync.dma_start_transpose(
                    out=T[:], in_=x2.ap()[base:base + cn, :])
                if g == 0:
                    # Weights are first needed by chunk 0's matmuls; issuing
                    # their DMA after the first transpose keeps the transpose
                    # at the head of the SP/HWDGE queue.
                    nc.sync.dma_start(out=w_sb[:], in_=w.ap())

                sq = spool.tile([128, cn], f32, tag="sq")
                nc.scalar.activation(sq[:], T[:], Square,
                                     bias=gbias[:, 0:1], scale=1.0 / H)
                bj = [None] * NB
                seed = bpool.tile([128, cn], bf16, tag="b0")
                nc.scalar.activation(seed[:], sq[:], Exp,
                                     bias=gbias[:, 1:2], scale=-1.0)
                bj[0] = seed
                r = bpool.tile([128, cn], bf16, tag="r")
                nc.scalar.activation(r[:], T[:], Exp,
                                     bias=gbias[:, 2:3], scale=2.0 / H)

                # Software-pipelined evicts, two chunks behind the activation
                # front so ScalarE never stalls waiting for matmul completion.
                if len(pending) >= 2:
                    flush_one()

                cd = 128 * int(round(DVE_FRAC * nblk))  # DVE stripe width
                for m in range(1, NB):
                    bt = bpool.tile([128, cn], bf16, tag=f"b{m}")
                    nc.vector.tensor_tensor(
                        bt[:, 0:cd], bj[m - 1][:, 0:cd], r[:, 0:cd], Mult)
                    if cd < cn:
                        nc.gpsimd.tensor_tensor(
                            bt[:, cd:cn], bj[m - 1][:, cd:cn],
                            r[:, cd:cn], Mult)
                    bj[m] = bt

                out_ps = opsum.tile([128, nblk * 2 * OUT], f32,
                                    tag=f"ps{g}", bufs=1)
                for k in range(nblk):
                    for j in range(NB):
                        nc.tensor.matmul(
                            out_ps[:, k * 2 * OUT:(k + 1) * 2 * OUT],
                            bj[j][:, k * 128:(k + 1) * 128],
                            w_sb[:, j * 2 * OUT:(j + 1) * 2 * OUT],
                            start=(j == 0),
                            stop=(j == NB - 1),
                        )
                pending.append((out_ps, nblk, base))
                base += cn

            while pending:
                flush_one()

    nc.compile()
    return nc


def _get_module():
    if "nc" not in _CACHE:
        _CACHE["nc"] = _build_module()
    return _CACHE["nc"]


def _build_w(coeffs_re, coeffs_im):
    import ml_dtypes

    # w2[p, j, o]: p = (part, i) contraction index, j = grid index within
    # part, o = (re outputs | im outputs). Scale folds the cascade constant
    # exp(-j^2 - C).
    w2 = np.empty((128, NB, 2 * OUT), dtype=np.float64)
    for j in range(NB):
        w2[:IN, j, :OUT] = coeffs_re[:, :, j]
        w2[:IN, j, OUT:] = coeffs_im[:, :, j]
        w2[IN:, j, :OUT] = coeffs_re[:, :, NB + j]
        w2[IN:, j, OUT:] = coeffs_im[:, :, NB + j]
        w2[:, j, :] *= np.exp(-float(j * j) - CBOOST)
    return w2.reshape(128, NB * 2 * OUT).astype(ml_dtypes.bfloat16)


def kernel(x_re, x_im, coeffs_re, coeffs_im, bias_re, bias_im):
    import ml_dtypes
    from concourse.bass_utils import run_bass_kernel_spmd

    nc = _get_module()
    w = _build_w(np.asarray(coeffs_re, dtype=np.float64),
                 np.asarray(coeffs_im, dtype=np.float64))
    x2 = np.concatenate(
        [np.asarray(x_re, dtype=np.float32),
         np.asarray(x_im, dtype=np.float32)], axis=1
    ).astype(ml_dtypes.bfloat16)

    in_maps = [
        {"x2": np.ascontiguousarray(x2[c * B_CORE:(c + 1) * B_CORE]), "w": w}
        for c in range(N_CORES)
    ]
    res = run_bass_kernel_spmd(nc, in_maps, core_ids=list(range(N_CORES)))
    out = np.empty((B, OUT), dtype=np.complex64)
    for c in range(N_CORES):
        yc = res.results[c]["y"]  # [B_CORE, 32] fp32
        out[c * B_CORE:(c + 1) * B_CORE] = yc[:, :OUT] + 1j * yc[:, OUT:]
    out += (np.asarray(bias_re) + 1j * np.asarray(bias_im)).astype(np.complex64)
    return out


# revision 16
# speedup vs baseline: 1.0372x; 1.0372x over previous
"""CVKAN layer Trainium2 kernel (v4: single-act-table Gaussian cascade).

Math (per reference):
    u = (x + 1)/h,  h = 2/(NB-1) = 2/7
    basis_j(x) = exp(-(u - j)^2)                  j = 0..7, per part (re|im)
    out[b, o]  = sum_{p=(part,i), j} basis_j(x_p[b]) * w[p, j, o] + bias[o]

Single-seed cascade, all activations from ONE table (exp_and_others:
Square, Exp, Copy) so there are zero activation-table reloads:
    sq    = Square(x/h + 1/h)          = u^2            (fp32)
    b'_0  = Exp(-sq + C)               = e^{C - u^2}    (bf16, C=30 boost)
    r     = Exp(2x/h + 2/h)            = e^{2u}         (bf16)
    b'_j  = b'_{j-1} * r                                (bf16 multiplies)
    basis_j = b'_j * e^{-j^2 - C}      (constant folded into the weights)

Device strategy (8 cores, pure batch data-parallel, no collectives):
  - Host concatenates re|im into x2 [B_CORE, 128] bf16; XBAR dma_start_transpose
    loads T [128, chunk] directly (no PE transposes).
  - ScalarE: 3 passes per chunk (sq, seed, r) + the PSUM->SBUF eviction copy.
  - 7 cascade multiplies per chunk, column-striped: DVE takes 13/16 of the
    columns (bf16 2x_1p mode ~0.52 ns/col), Pool/GpSimd the rest.
  - PE: j-major accumulating matmuls with the basis block [128,128] as the
    STATIONARY operand and W_j [128, 32] moving: 32 output rows per matmul,
    and after the last cascade link only the j=7 matmuls remain (short tail).
    PSUM collects out[b, o] directly - no output transpose.
  - Evictions are software-pipelined one chunk behind the activations so the
    ScalarE queue never stalls on matmul completion.
"""

import sys

import numpy as np

if "/opt/trn_rl_repo" not in sys.path:
    sys.path.append("/opt/trn_rl_repo")

B = 65536
IN = 64
OUT = 16
NB = 8
N_CORES = 8
B_CORE = B // N_CORES  # 8192
H = 2.0 / (NB - 1)
CBOOST = 30.0

# Graduated chunk sizes (batch rows per pipeline stage). nblk = cn/128 must
# stay <= 16 so each chunk's PSUM tile [128, nblk*32] f32 fits one 2KB bank.
# Small first chunk shortens the pipeline fill (T-DMA -> sq -> seed -> links);
# smaller last chunk shortens the cascade->evict->store tail.
CHUNKS = [512, 2048, 2048, 2048, 1536]
assert sum(CHUNKS) == B_CORE
# Fraction of link columns the DVE takes (rest on Pool), in 128-col blocks.
DVE_FRAC = 13.0 / 16.0

_CACHE = {}


def _build_module():
    import concourse.mybir as mybir
    import concourse.tile as tile
    from concourse import bacc

    f32 = mybir.dt.float32
    bf16 = mybir.dt.bfloat16
    nc = bacc.Bacc("TRN2", target_bir_lowering=False, debug=False,
                   num_devices=N_CORES)

    x2 = nc.dram_tensor("x2", [B_CORE, 128], bf16, kind="ExternalInput")
    w = nc.dram_tensor("w", [128, NB * 2 * OUT], bf16, kind="ExternalInput")
    y = nc.dram_tensor("y", [B_CORE, 2 * OUT], f32, kind="ExternalOutput")

    Square = mybir.ActivationFunctionType.Square
    Exp = mybir.ActivationFunctionType.Exp
    Copy = mybir.ActivationFunctionType.Copy
    Mult = mybir.AluOpType.mult

    with tile.TileContext(nc) as tc:
        with (
            tc.tile_pool(name="consts", bufs=1) as consts,
            tc.tile_pool(name="tin", bufs=len(CHUNKS)) as tpool,
            tc.tile_pool(name="sq", bufs=2) as spool,
            tc.tile_pool(name="basis", bufs=2) as bpool,
            tc.tile_pool(name="opsum", bufs=1, space="PSUM") as opsum,
            tc.tile_pool(name="osb", bufs=2) as opool,
        ):
            # Per-partition bias columns (floats need pre-registered consts).
            gbias = consts.tile([128, 4], f32)
            nc.vector.memset(gbias[:, 0:1], 1.0 / H)
            nc.vector.memset(gbias[:, 1:2], CBOOST)
            nc.vector.memset(gbias[:, 2:3], 2.0 / H)
            nc.vector.memset(gbias[:, 3:4], 0.0)
            # Dummy activation: forces the one-and-only act-table load to run
            # at t~0, overlapped with the first input DMA, instead of sitting
            # on the critical path of the first chunk's activations.
            warm = consts.tile([128, 1], bf16)
            nc.scalar.activation(warm[:], gbias[:, 3:4], Exp,
                                 bias=gbias[:, 3:4])
            # Weight load issued from the Activation engine's DMA path so the
            # SP queue holds only the input transposes (a DMA's sem waits
            # block its whole issue queue).
            w_sb = consts.tile([128, NB * 2 * OUT], bf16)
            nc.scalar.dma_start(out=w_sb[:], in_=w.ap())

            # Pre-issue every input transpose: T tiles are single-use
            # (bufs=len(CHUNKS)), so none of these DMAs carries a wait and
            # the SP queue streams them back to back from t=0.
            t_tiles = []
            base = 0
            for g, cn in enumerate(CHUNKS):
                T = tpool.tile([128, cn], bf16, tag=f"T{g}")
                nc.sync.dma_start_transpose(
                    out=T[:], in_=x2.ap()[base:base + cn, :])
                t_tiles.append(T)
                base += cn

            pending = []  # (psum, nblk, base) awaiting evict + store

            def flush_one():
                ps, nblk, base0 = pending.pop(0)
                out_sb = opool.tile([128, nblk * 2 * OUT], f32, tag="out_sb")
                nc.scalar.activation(out_sb[:], ps[:], Copy)
                nc.sync.dma_start(
                    out=y.ap()[base0:base0 + nblk * 128, :]
                        .rearrange("(g p) o -> p g o", p=128),
                    in_=out_sb[:].rearrange("p (g o) -> p g o", g=nblk),
                )

            base = 0
            for g, cn in enumerate(CHUNKS):
                nblk = cn // 128
                T = t_tiles[g]
                sq = spool.tile([128, cn], f32, tag="sq")
                nc.scalar.activation(sq[:], T[:], Square,
                                     bias=gbias[:, 0:1], scale=1.0 / H)
                bj = [None] * NB
                seed = bpool.tile([128, cn], bf16, tag="b0")
                nc.scalar.activation(seed[:], sq[:], Exp,
                                     bias=gbias[:, 1:2], scale=-1.0)
                bj[0] = seed
                r = bpool.tile([128, cn], bf16, tag="r")
                nc.scalar.activation(r[:], T[:], Exp,
                                     bias=gbias[:, 2:3], scale=2.0 / H)

                # Software-pipelined evicts, two chunks behind the activation
                # front so ScalarE never stalls waiting for matmul completion.
                if len(pending) >= 2:
                    flush_one()

                cd = 128 * int(round(DVE_FRAC * nblk))  # DVE stripe width
                for m in range(1, NB):
                    bt = bpool.tile([128, cn], bf16, tag=f"b{m}")
                    nc.vector.tensor_tensor(
                        bt[:, 0:cd], bj[m - 1][:, 0:cd], r[:, 0:cd], Mult)
                    if cd < cn:
                        nc.gpsimd.tensor_tensor(
                            bt[:, cd:cn], bj[m - 1][:, cd:cn],
                            r[:, cd:cn], Mult)
                    bj[m] = bt

                out_ps = opsum.tile([128, nblk * 2 * OUT], f32,
                                    tag=f"ps{g}", bufs=1)
                for k in range(nblk):
                    for j in range(NB):
                        nc.tensor.matmul(
                            out_ps[:, k * 2 * OUT:(k + 1) * 2 * OUT],
                            bj[j][:, k * 128:(k + 1) * 128],
                            w_sb[:, j * 2 * OUT:(j + 1) * 2 * OUT],
                            start=(j == 0),
                            stop=(j == NB - 1),
                        )
                pending.append((out_ps, nblk, base))
                base += cn

            while pending:
                flush_one()

    nc.compile()
    return nc


def _get_module():
    if "nc" not in _CACHE:
        _CACHE["nc"] = _build_module()
    return _CACHE["nc"]


def _build_w(coeffs_re, coeffs_im):
    import ml_dtypes

    # w2[p, j, o]: p = (part, i) contraction index, j = grid index within
    # part, o = (re outputs | im outputs). Scale folds the cascade constant
    # exp(-j^2 - C).
    w2 = np.empty((128, NB, 2 * OUT), dtype=np.float64)
    for j in range(NB):
        w2[:IN, j, :OUT] = coeffs_re[:, :, j]
        w2[:IN, j, OUT:] = coeffs_im[:, :, j]
        w2[IN:, j, :OUT] = coeffs_re[:, :, NB + j]
        w2[IN:, j, OUT:] = coeffs_im[:, :, NB + j]
        w2[:, j, :] *= np.exp(-float(j * j) - CBOOST)
    return w2.reshape(128, NB * 2 * OUT).astype(ml_dtypes.bfloat16)


def kernel(x_re, x_im, coeffs_re, coeffs_im, bias_re, bias_im):
    import ml_dtypes
    from concourse.bass_utils import run_bass_kernel_spmd

    nc = _get_module()
    w = _build_w(np.asarray(coeffs_re, dtype=np.float64),
                 np.asarray(coeffs_im, dtype=np.float64))
    x2 = np.concatenate(
        [np.asarray(x_re, dtype=np.float32),
         np.asarray(x_im, dtype=np.float32)], axis=1
    ).astype(ml_dtypes.bfloat16)

    in_maps = [
        {"x2": np.ascontiguousarray(x2[c * B_CORE:(c + 1) * B_CORE]), "w": w}
        for c in range(N_CORES)
    ]
    res = run_bass_kernel_spmd(nc, in_maps, core_ids=list(range(N_CORES)))
    out = np.empty((B, OUT), dtype=np.complex64)
    for c in range(N_CORES):
        yc = res.results[c]["y"]  # [B_CORE, 32] fp32
        out[c * B_CORE:(c + 1) * B_CORE] = yc[:, :OUT] + 1j * yc[:, OUT:]
    out += (np.asarray(bias_re) + 1j * np.asarray(bias_im)).astype(np.complex64)
    return out
